# revision 1
# baseline (speedup 1.0000x reference)
"""Trainium2 Bass kernel for nn_BondPredictor (gnn_message_passing).

Computes, for each batch b:
    A      = hidden_states[b][clip(atom_indices[b])]          # [256, 512] gather
    pair   = concat(A[i]+A[j], |A[i]-A[j]|)                   # [256,256,1024]
    h      = gelu(pair @ W1 + b1)                             # [256,256,512]
    logits = h @ W2 + b2  -> [7, 256, 256], diagonal = -10000

Sharding: 8 cores = 2 batches x 4 row-blocks of 64 rows. Each core's atom
axis is ROLLED by -64*(c%4) so every core computes rows 0..63 of its rolled
grid with an identical program (pure SPMD); the host un-rolls the output
columns when unsharding.

Symmetry: pair(i,j) == pair(j,i) exactly, so each row only computes the
cyclic half-window of columns (j-i) mod 256 in [0,128] (129 values: 128 in
the quad loop + 1 antipodal column); the host mirrors offsets 129..255 from
the transpose during unshard. This halves all per-pair compute.

Algebraic split: (A[i]+A[j]) @ W1a = P[i] + P[j] with P = A @ W1a computed
once per core -> only the |A[i]-A[j]| half needs the big per-pair matmul,
and with |d| = 2*relu(d) - d the per-pair contraction uses relu features
(a valid subtract+max DVE dual-op) while the linear -d part folds into
Q = A @ W1b. The per-pair rank-one terms (P-Q)[j] + (P+Q+b1)[i] enter PSUM
through an identity-weight matmul; b2 rides the PSUM->SBUF copy; the
diagonal -10000 fill is a single affine_select per row-quad on GPSIMD.
All matmuls bf16 with fp32 PSUM accumulation; exact erf-GELU on ScalarE.
"""

import sys

sys.path.insert(0, "/opt/trn_rl_repo")

import numpy as np
import ml_dtypes

B, T, D, N, C = 2, 1024, 512, 256, 7
NCORES = 8
RB = 4                # row-blocks per batch
NL = N // RB          # 64 rows per core
QR = 4                # rows per quad
NQ = NL // QR         # 16 quads
KC = D // 128         # 4 chunks of the 512-dim contraction
TC_ = T // 128        # 8 chunks of the sequence dim
TW = 128              # cols per row: cyclic offsets (j-i) mod N in [0,127];
                      # offset 128 is the extra antipodal column; offsets
                      # 129..255 come from the exact grid symmetry (mirror)
MASK_FILL = -10000.0

_CACHE = {}


def _build(reps=1):
    """Build + compile the per-core Bass program. Returns (nc, names)."""
    import concourse.bass as bass
    import concourse.bacc as bacc
    import concourse.tile as tile
    from concourse import mybir

    f32 = mybir.dt.float32
    bf16 = mybir.dt.bfloat16
    i32 = mybir.dt.int32
    Alu = mybir.AluOpType
    Act = mybir.ActivationFunctionType

    nc = bacc.Bacc("TRN2", target_bir_lowering=False, debug=False)

    h_d = nc.dram_tensor("h", [T, D], bf16, kind="ExternalInput")
    idx_d = nc.dram_tensor("idxf", [1, N], f32, kind="ExternalInput")
    w1_d = nc.dram_tensor("w1", [2 * D, D], bf16, kind="ExternalInput")
    w2_d = nc.dram_tensor("w2", [D, C], bf16, kind="ExternalInput")
    b1_d = nc.dram_tensor("b1", [D, 1], f32, kind="ExternalInput")
    b2_d = nc.dram_tensor("b2", [C, 1], f32, kind="ExternalInput")
    out_d = nc.dram_tensor("out", [C, NL, TW + 1], f32, kind="ExternalOutput")

    h_ap, idx_ap = h_d.ap(), idx_d.ap()
    w1_ap, w2_ap = w1_d.ap(), w2_d.ap()
    b1_ap, b2_ap, out_ap = b1_d.ap(), b2_d.ap(), out_d.ap()

    with tile.TileContext(nc) as tc:
        from contextlib import ExitStack

        with ExitStack() as ctx:
            const = ctx.enter_context(tc.tile_pool(name="const", bufs=1))
            wpool = ctx.enter_context(tc.tile_pool(name="w", bufs=1))
            gpool = ctx.enter_context(tc.tile_pool(name="g", bufs=2))
            work = ctx.enter_context(tc.tile_pool(name="work", bufs=4))
            opool = ctx.enter_context(tc.tile_pool(name="o", bufs=3))
            ph = ctx.enter_context(
                tc.tile_pool(name="ph", bufs=4, space=bass.MemorySpace.PSUM)
            )
            po = ctx.enter_context(
                tc.tile_pool(name="po", bufs=3, space=bass.MemorySpace.PSUM)
            )

            # ---- one-time constants (outside rep loop) ----
            ones1 = const.tile([1, 128], f32, tag="ones1")
            nc.vector.memset(ones1[:], 1.0)
            onesq = const.tile([128, 128], bf16, tag="onesq")
            nc.vector.memset(onesq[:], 1.0)
            ident = const.tile([128, 128], bf16, tag="ident")
            # iota[p,f] = p - f -> ==0 on the diagonal
            nc.gpsimd.affine_select(
                ident[:], onesq[:], pattern=[[-1, 128]],
                compare_op=Alu.is_equal, fill=0.0, base=0, channel_multiplier=1,
            )
            iota_i = const.tile([128, TC_], i32, tag="iota_i")
            nc.gpsimd.iota(iota_i[:], pattern=[[128, TC_]], base=0, channel_multiplier=1)
            iota_f = const.tile([128, TC_], f32, tag="iota_f")
            nc.vector.tensor_copy(iota_f[:], iota_i[:])

            b1c = const.tile([128, KC], f32, tag="b1c")
            for m in range(KC):
                nc.sync.dma_start(b1c[:, m : m + 1], b1_ap[128 * m : 128 * (m + 1), :])
            b2c = const.tile([C, 1], f32, tag="b2c")
            nc.sync.dma_start(b2c[:], b2_ap[:])

            # weights: w1 row-chunks [128, 512]; rows 0..511 = W1a, 512..1023 = W1b
            w1sb = []
            for k in range(2 * KC):
                t = wpool.tile([128, D], bf16, tag=f"w1_{k}")
                nc.sync.dma_start(t[:], w1_ap[128 * k : 128 * (k + 1), :])
                w1sb.append(t)
            w2c = []
            for m in range(KC):
                t = wpool.tile([128, C], bf16, tag=f"w2_{m}")
                nc.sync.dma_start(t[:], w2_ap[128 * m : 128 * (m + 1), :])
                w2c.append(t)

            def body():
                # ---- load hidden rows ----
                ht = []
                for t_ in range(TC_):
                    tl = gpool.tile([128, D], bf16, tag=f"ht_{t_}")
                    nc.sync.dma_start(tl[:], h_ap[128 * t_ : 128 * (t_ + 1), :])
                    ht.append(tl)

                # ---- phase A: one-hot of idx, [t, n] layout ----
                idx_sb = gpool.tile([1, N], f32, tag="idx_sb")
                nc.sync.dma_start(idx_sb[:], idx_ap[:])
                ps_i = po.tile([128, N], f32, tag="po")
                nc.tensor.matmul(ps_i[:], ones1[:], idx_sb[:])  # bcast idx to 128 parts
                idxb = gpool.tile([128, N], f32, tag="idxb")
                nc.vector.tensor_copy(idxb[:], ps_i[:])
                oh = []
                for t_ in range(TC_):
                    o = gpool.tile([128, N], bf16, tag=f"oh_{t_}")
                    nc.vector.tensor_scalar(
                        o[:], idxb[:], iota_f[:, t_ : t_ + 1], None, op0=Alu.is_equal
                    )
                    oh.append(o)

                # ---- phase B: gather A_T = H_T @ onehot ----
                # |d| = 2*relu(d) - d : the per-pair matmul only needs
                # rp = relu(2*x_j - 2*x_i) (valid subtract+max dual op); the
                # linear -d part folds into Qmat = A @ W1b rank-one terms.
                at_bf, at2_bf, at32_2 = [], [], []
                for m in range(KC):
                    ps_g = po.tile([128, N], f32, tag="po")
                    for t_ in range(TC_):
                        nc.tensor.matmul(
                            ps_g[:],
                            ht[t_][:, 128 * m : 128 * (m + 1)],
                            oh[t_][:],
                            start=(t_ == 0),
                            stop=(t_ == TC_ - 1),
                        )
                    a_bf = gpool.tile([128, N], bf16, tag=f"at_bf_{m}")
                    nc.vector.tensor_copy(a_bf[:], ps_g[:])
                    # doubled [A|A] so cyclic column windows are contiguous
                    a2_bf = gpool.tile([128, 2 * N], bf16, tag=f"at2x_{m}")
                    nc.vector.tensor_scalar(
                        a2_bf[:, 0:N], ps_g[:], 2.0, None, op0=Alu.mult
                    )
                    nc.vector.tensor_scalar(
                        a2_bf[:, N : 2 * N], ps_g[:], 2.0, None, op0=Alu.mult
                    )
                    a2_32 = gpool.tile([128, NL], f32, tag=f"at32_2_{m}")
                    nc.vector.tensor_scalar(
                        a2_32[:], ps_g[:, 0:NL], 2.0, None, op0=Alu.mult
                    )
                    at_bf.append(a_bf)
                    at2_bf.append(a2_bf)
                    at32_2.append(a2_32)

                # ---- phase C: Pmat = A@W1a, Qmat = A@W1b (transposed layouts) ----
                pm32, qm32 = [], []
                for m in range(KC):
                    ps_p = po.tile([128, N], f32, tag="po")
                    for k in range(KC):
                        nc.tensor.matmul(
                            ps_p[:],
                            w1sb[k][:, 128 * m : 128 * (m + 1)],
                            at_bf[k][:],
                            start=(k == 0),
                            stop=(k == KC - 1),
                        )
                    p_32 = gpool.tile([128, N], f32, tag=f"pm32_{m}")
                    nc.vector.tensor_copy(p_32[:], ps_p[:])
                    pm32.append(p_32)
                for m in range(KC):
                    ps_q = po.tile([128, N], f32, tag="po")
                    for k in range(KC):
                        nc.tensor.matmul(
                            ps_q[:],
                            w1sb[KC + k][:, 128 * m : 128 * (m + 1)],
                            at_bf[k][:],
                            start=(k == 0),
                            stop=(k == KC - 1),
                        )
                    q_32 = gpool.tile([128, N], f32, tag=f"qm32_{m}")
                    nc.vector.tensor_copy(q_32[:], ps_q[:])
                    qm32.append(q_32)
                # PmQ = Pmat - Qmat (j-term), PpQb = Pmat + Qmat + b1 (i-term)
                pmq_bf, ppqb32 = [], []
                for m in range(KC):
                    d_bf = gpool.tile([128, 2 * N], bf16, tag=f"pmq2x_{m}")
                    nc.vector.tensor_tensor(
                        d_bf[:, 0:N], pm32[m][:], qm32[m][:], op=Alu.subtract
                    )
                    nc.vector.tensor_tensor(
                        d_bf[:, N : 2 * N], pm32[m][:], qm32[m][:], op=Alu.subtract
                    )
                    s_32 = gpool.tile([128, NL], f32, tag=f"ppqb32_{m}")
                    nc.vector.scalar_tensor_tensor(
                        s_32[:],
                        pm32[m][:, 0:NL],
                        b1c[:, m : m + 1],
                        qm32[m][:, 0:NL],
                        op0=Alu.add,
                        op1=Alu.add,
                    )
                    pmq_bf.append(d_bf)
                    ppqb32.append(s_32)

                # ---- main loop over row-quads: row i covers cyclic cols
                # j = i..i+127 (the symmetric half of the grid) ----
                for q in range(NQ):
                    absq = work.tile([128, KC * QR * TW], bf16, tag="absq")
                    pp = work.tile([128, KC * QR * TW], bf16, tag="pp")
                    for k in range(KC):
                        for r in range(QR):
                            i = QR * q + r
                            nc.vector.tensor_scalar(
                                absq[:, 512 * k + TW * r : 512 * k + TW * (r + 1)],
                                at2_bf[k][:, i : i + TW],
                                at32_2[k][:, i : i + 1],
                                0.0,
                                op0=Alu.subtract,
                                op1=Alu.max,
                            )
                    for m in range(KC):
                        for r in range(QR):
                            i = QR * q + r
                            nc.vector.tensor_scalar(
                                pp[:, 512 * m + TW * r : 512 * m + TW * (r + 1)],
                                pmq_bf[m][:, i : i + TW],
                                ppqb32[m][:, i : i + 1],
                                None,
                                op0=Alu.add,
                            )

                    hh = work.tile([128, KC * 512], bf16, tag="hh")
                    for m in range(KC):
                        ps_h = ph.tile([128, 512], f32, tag="ph")
                        # P/Q rank-one terms via identity weights; start=True
                        # zeroes the 2KB bank region
                        nc.tensor.matmul(
                            ps_h[:],
                            ident[:],
                            pp[:, 512 * m : 512 * (m + 1)],
                            start=True,
                            stop=False,
                        )
                        for k in range(KC):
                            nc.tensor.matmul(
                                ps_h[:],
                                w1sb[KC + k][:, 128 * m : 128 * (m + 1)],
                                absq[:, 512 * k : 512 * (k + 1)],
                                start=False,
                                stop=(k == KC - 1),
                            )
                        nc.scalar.activation(
                            hh[:, 512 * m : 512 * (m + 1)], ps_h[:], Act.Gelu
                        )

                    ps_o = po.tile([C, 512], f32, tag="po")
                    for m in range(KC):
                        nc.tensor.matmul(
                            ps_o[:],
                            w2c[m][:],
                            hh[:, 512 * m : 512 * (m + 1)],
                            start=(m == 0),
                            stop=(m == KC - 1),
                        )
                    tmp = opool.tile([C, 512], f32, tag="tmp")
                    nc.vector.tensor_scalar(
                        tmp[:], ps_o[:], b2c[:], None, op0=Alu.add
                    )
                    outq = opool.tile([C, 512], f32, tag="outq")
                    # col t==0 of each row block is j==i: the diagonal
                    nc.gpsimd.affine_select(
                        outq[:], tmp[:], pattern=[[0, QR], [1, TW]],
                        compare_op=Alu.not_equal, fill=MASK_FILL,
                        base=0, channel_multiplier=0,
                    )
                    nc.sync.dma_start(out_ap[:, QR * q : QR * (q + 1), 0:TW], outq[:])

                # ---- antipodal pass: pairs (i, i+128), offset not covered
                # by the half-window nor by the mirror ----
                rpA = work.tile([128, KC * NL], bf16, tag="rpA")
                ppA = work.tile([128, KC * NL], bf16, tag="ppA")
                for k in range(KC):
                    dA = work.tile([128, NL], f32, tag="dA")
                    nc.vector.tensor_tensor(
                        dA[:], at2_bf[k][:, TW : TW + NL], at2_bf[k][:, 0:NL],
                        op=Alu.subtract,
                    )
                    nc.vector.tensor_scalar(
                        rpA[:, NL * k : NL * (k + 1)], dA[:], 0.0, None, op0=Alu.max
                    )
                for m in range(KC):
                    nc.vector.tensor_tensor(
                        ppA[:, NL * m : NL * (m + 1)],
                        pmq_bf[m][:, TW : TW + NL],
                        ppqb32[m][:],
                        op=Alu.add,
                    )
                hhA = work.tile([128, KC * NL], bf16, tag="hhA")
                for m in range(KC):
                    psA = ph.tile([128, NL], f32, tag="ph")
                    nc.tensor.matmul(
                        psA[:], ident[:], ppA[:, NL * m : NL * (m + 1)],
                        start=True, stop=False,
                    )
                    for k in range(KC):
                        nc.tensor.matmul(
                            psA[:],
                            w1sb[KC + k][:, 128 * m : 128 * (m + 1)],
                            rpA[:, NL * k : NL * (k + 1)],
                            start=False,
                            stop=(k == KC - 1),
                        )
                    nc.scalar.activation(
                        hhA[:, NL * m : NL * (m + 1)], psA[:], Act.Gelu
                    )
                psoA = po.tile([C, NL], f32, tag="po")
                for m in range(KC):
                    nc.tensor.matmul(
                        psoA[:], w2c[m][:], hhA[:, NL * m : NL * (m + 1)],
                        start=(m == 0), stop=(m == KC - 1),
                    )
                tmpA = opool.tile([C, NL], f32, tag="tmpA")
                nc.vector.tensor_scalar(tmpA[:], psoA[:], b2c[:], None, op0=Alu.add)
                nc.sync.dma_start(out_ap[:, :, TW : TW + 1], tmpA[:])

            for _ in range(reps):
                body()

    nc.compile()
    return nc


def _get(reps=1):
    if reps not in _CACHE:
        _CACHE[reps] = _build(reps)
    return _CACHE[reps]


def _shard_inputs(hidden_states, W1, b1, W2, b2, atom_indices):
    hs = np.asarray(hidden_states, np.float32)
    idx = np.clip(np.asarray(atom_indices).astype(np.int64), 0, T - 1)
    w1b = np.asarray(W1, np.float32).astype(ml_dtypes.bfloat16)
    w2b = np.asarray(W2, np.float32).astype(ml_dtypes.bfloat16)
    b1f = np.asarray(b1, np.float32).reshape(D, 1)
    b2f = np.asarray(b2, np.float32).reshape(C, 1)
    in_maps = []
    for c in range(NCORES):
        b = c // RB
        r0 = NL * (c % RB)
        idx_roll = np.roll(idx[b], -r0).astype(np.float32).reshape(1, N)
        in_maps.append(
            {
                "h": hs[b].astype(ml_dtypes.bfloat16),
                "idxf": idx_roll,
                "w1": w1b,
                "w2": w2b,
                "b1": b1f,
                "b2": b2f,
            }
        )
    return in_maps


def _unshard(results, atom_mask):
    full = np.empty((B, C, N, N), np.float32)
    for c in range(NCORES):
        b = c // RB
        r0 = NL * (c % RB)
        blk = results[c]["out"]  # [C, 64, 129]: row i -> cols (i+t)%N, t=0..128
        rows = r0 + np.arange(NL)
        idx_j = (rows[:, None] + np.arange(TW + 1)[None, :]) % N  # [64, 129]
        np.put_along_axis(
            full[b, :, r0 : r0 + NL, :],
            np.broadcast_to(idx_j[None], (C, NL, TW + 1)),
            blk,
            axis=2,
        )
    # grid symmetry: logits[i,j] == logits[j,i]; offsets 129..255 mirror
    offs = (np.arange(N)[None, :] - np.arange(N)[:, None]) % N
    low = offs > TW
    fullT = np.transpose(full, (0, 1, 3, 2))
    full = np.where(low[None, None], fullT, full)
    mask = np.asarray(atom_mask).astype(bool)
    if not mask.all():
        valid = mask[:, :, None] & mask[:, None, :]
        valid &= ~np.eye(N, dtype=bool)[None]
        full = np.where(valid[:, None, :, :], full, np.float32(MASK_FILL))
    return full


def kernel(hidden_states, W1, b1, W2, b2, atom_indices, atom_mask):
    from concourse.bass_utils import run_bass_kernel_spmd

    nc = _get(1)
    in_maps = _shard_inputs(hidden_states, W1, b1, W2, b2, atom_indices)
    res = run_bass_kernel_spmd(nc, in_maps, list(range(NCORES)))
    return _unshard(res.results, atom_mask)



# revision 3
# speedup vs baseline: 2.0741x; 2.0741x over previous
"""Trainium2 Bass kernel for nn_BondPredictor (gnn_message_passing) — v3.

Computes, for each batch b:
    A      = hidden_states[b][clip(atom_indices[b])]          # [256, 512] gather
    pair   = concat(A[i]+A[j], |A[i]-A[j]|)                   # [256,256,1024]
    h      = gelu(pair @ W1 + b1)                             # [256,256,512]
    logits = h @ W2 + b2  -> [7, 256, 256], diagonal = -10000

Sharding: 8 cores = 2 batches x 4 row-blocks of 64 rows; atom axis rolled by
-64*(c%4) per core (pure SPMD); host un-rolls on unshard. Pair symmetry:
each row computes cyclic offsets (j-i) mod 256 in [0,128]; the host mirrors
offsets 129..255 from the transpose.

v3 engine plan (vs the bf16 v2 baseline at ~130us):
- Gather via indirect DMA + DMA transposes (no one-hot matmuls).
- |d| = 2 relu(d) - d; relu features quantized to fp8e4 by DVE dual-op
  tensor_scalar (sub,max) at the 2x SBUF rate; the per-pair contraction runs
  as fp8 DoubleRow matmuls (2x PE), first-layer weights prescaled x128 on
  the host (x64 fp8-range scale, x2 from the relu identity); the gelu
  activation applies scale=1/64 to descale.
- Rank-one injections ws*(P-Q)[j] and ws*(P+Q+b1)[i] enter PSUM through
  identity-weight DoubleRow matmuls whose moving operand is a 4-D strided
  AP (overlapping column windows for the j-term; inner stride-0 broadcast
  for the i-term). Zero per-quad DVE cost for the rank-one terms.
- P/Q phase also fp8 DoubleRow with host-prepped ws*(W1a-+W1b) weights.
- Second layer bf16; outputs of 3 quads packed into one PSUM bank at
  partition offsets 0/32/64 so the b2-add drain is one DVE op per 3 quads.
- Diagonal fill and the symmetry mirror happen on the host.
"""

import sys

sys.path.insert(0, "/opt/trn_rl_repo")

import numpy as np
import ml_dtypes

F8 = ml_dtypes.float8_e4m3
BF = ml_dtypes.bfloat16

B, T, D, N, C = 2, 1024, 512, 256, 7
NCORES = 8
RB = 4                # row-blocks per batch
NL = N // RB          # 64 rows per core
QR = 4                # rows per quad
NQ = NL // QR         # 16 quads
KC = D // 128         # 4 chunks of the 512-dim contraction
TW = 128              # cols per row: cyclic offsets (j-i) in [0,127]
MASK_FILL = -10000.0
WS = 64.0             # fp8 weight prescale

POOL_CHUNKS = 1       # absq feature-chunks whose subtract runs on GpSimd

_CACHE = {}


def _build(reps=1, sim_compat=False):
    import concourse.bass as bass
    import concourse.bacc as bacc
    import concourse.tile as tile
    from concourse import mybir

    f32 = mybir.dt.float32
    bf16 = mybir.dt.bfloat16
    fp8 = mybir.dt.float8e4
    i32 = mybir.dt.int32
    Alu = mybir.AluOpType
    Act = mybir.ActivationFunctionType
    DR = mybir.MatmulPerfMode.DoubleRow
    ACTF = Act.Relu if sim_compat else Act.Gelu

    nc = bacc.Bacc("TRN2", target_bir_lowering=False, debug=False)

    h_d = nc.dram_tensor("h", [T, D], bf16, kind="ExternalInput")
    idx_d = nc.dram_tensor("idx", [128, 2], i32, kind="ExternalInput")
    w1b8_d = nc.dram_tensor("w1b8", [D, D], fp8, kind="ExternalInput")
    wpm8_d = nc.dram_tensor("wpm8", [D, D], fp8, kind="ExternalInput")
    wpp8_d = nc.dram_tensor("wpp8", [D, D], fp8, kind="ExternalInput")
    w2b_d = nc.dram_tensor("w2b", [128, KC * C], bf16, kind="ExternalInput")
    b1s_d = nc.dram_tensor("b1s", [128, KC], f32, kind="ExternalInput")
    b2r_d = nc.dram_tensor("b2r", [128, 1], f32, kind="ExternalInput")
    id8_d = nc.dram_tensor("id8", [128, 256], fp8, kind="ExternalInput")
    out1_d = nc.dram_tensor("out1", [NQ, C, QR * TW], f32, kind="ExternalOutput")
    out2_d = nc.dram_tensor("out2", [C, NL], f32, kind="ExternalOutput")

    with tile.TileContext(nc) as tc:
        from contextlib import ExitStack

        with ExitStack() as ctx:
            const = ctx.enter_context(tc.tile_pool(name="const", bufs=1))
            gpool = ctx.enter_context(tc.tile_pool(name="g", bufs=2))
            work = ctx.enter_context(tc.tile_pool(name="work", bufs=3))
            opool = ctx.enter_context(tc.tile_pool(name="o", bufs=2))
            # PSUM: ph 1 tag x 3 bufs x 2 banks + po 1 tag x 2 bufs x 1 bank = 8
            ph = ctx.enter_context(
                tc.tile_pool(name="ph", bufs=3, space=bass.MemorySpace.PSUM)
            )
            po = ctx.enter_context(
                tc.tile_pool(name="po", bufs=2, space=bass.MemorySpace.PSUM)
            )

            # ---- one-time constants ----
            id8 = const.tile([128, 256], fp8, tag="id8")
            nc.sync.dma_start(id8[:], id8_d.ap())
            id8_3d = id8[:].rearrange("p (s m) -> p s m", s=2)
            idb = const.tile([128, 128], bf16, tag="idb")
            nc.vector.tensor_copy(idb[:], id8[:, 0:128])

            def slot(ap_, sl):
                dims = [list(d) for d in ap_.ap]
                return bass.AP(
                    tensor=ap_.tensor,
                    offset=ap_.offset + sl * dims[1][0],
                    ap=[dims[0]] + dims[2:],
                )

            def mm_dr(out, lhsT, rhs, start, stop):
                """DoubleRow matmul; in sim_compat, lower to interp-friendly
                non-DR matmuls (slot loop) with identical operands/deps."""
                if not sim_compat:
                    nc.tensor.matmul(out, lhsT, rhs, start=start, stop=stop,
                                     perf_mode=DR)
                    return
                for sl in range(2):
                    nc.tensor.matmul(out, slot(lhsT, sl), slot(rhs, sl),
                                     start=(start and sl == 0),
                                     stop=(stop and sl == 1))
            w1b8 = const.tile([128, KC * 512], fp8, tag="w1b8")
            wpm8 = const.tile([128, KC * 512], fp8, tag="wpm8")
            wpp8 = const.tile([128, KC * 512], fp8, tag="wpp8")
            for m in range(KC):
                sl = slice(512 * m, 512 * (m + 1))
                rows = slice(128 * m, 128 * (m + 1))
                nc.sync.dma_start(w1b8[:, sl], w1b8_d.ap()[rows, :])
                nc.sync.dma_start(wpm8[:, sl], wpm8_d.ap()[rows, :])
                nc.sync.dma_start(wpp8[:, sl], wpp8_d.ap()[rows, :])
            w2sb = const.tile([128, KC * C], bf16, tag="w2sb")
            nc.sync.dma_start(w2sb[:], w2b_d.ap())
            b1s = const.tile([128, KC], f32, tag="b1s")
            nc.sync.dma_start(b1s[:], b1s_d.ap())
            b2r = const.tile([128, 1], f32, tag="b2r")
            nc.sync.dma_start(b2r[:], b2r_d.ap())

            def wtile(t, m):
                return t[:, 512 * m : 512 * (m + 1)].rearrange(
                    "p (k f) -> p k f", k=KC
                )

            def body():
                # ---- gather A = h[idx] (atom-major), transpose to f-major ----
                idx_sb = gpool.tile([128, 2], i32, tag="idx_sb")
                nc.sync.dma_start(idx_sb[:], idx_d.ap())
                ga = []
                for t_ in range(2):
                    g = gpool.tile([128, D], bf16, tag=f"ga{t_}")
                    nc.gpsimd.indirect_dma_start(
                        out=g[:], out_offset=None, in_=h_d.ap(),
                        in_offset=bass.IndirectOffsetOnAxis(
                            ap=idx_sb[:, t_ : t_ + 1], axis=0
                        ),
                    )
                    ga.append(g)
                at = gpool.tile([128, KC, N], bf16, tag="at")
                for t_ in range(2):
                    for k in range(KC):
                        nc.sync.dma_start_transpose(
                            at[:, k, 128 * t_ : 128 * (t_ + 1)],
                            ga[t_][:, 128 * k : 128 * (k + 1)],
                        )
                at8 = gpool.tile([128, KC, N], fp8, tag="at8")
                nc.vector.tensor_copy(
                    at8[:].rearrange("p k a -> p (k a)"),
                    at[:].rearrange("p k a -> p (k a)"),
                )
                ai32 = gpool.tile([128, KC, NL], f32, tag="ai32")
                nc.vector.tensor_copy(
                    ai32[:].rearrange("p k a -> p (k a)"), at[:, :, 0:NL]
                )

                # ---- P/Q phase: p2 = ws(P-Q) all atoms, pqb8 = ws(P+Q+b1)[0:64]
                p2, pqb8 = [], []
                for m in range(KC):
                    ps_c = po.tile([128, 512], f32, tag="po")
                    for kk in range(2):
                        mm_dr(
                            ps_c[:, 0:N],
                            wtile(wpm8, m)[:, 2 * kk : 2 * kk + 2, :],
                            at8[:, 2 * kk : 2 * kk + 2, :],
                            start=(kk == 0), stop=(kk == 1),
                        )
                    p = gpool.tile([128, N], fp8, tag=f"p2_{m}")
                    nc.vector.tensor_copy(p[:], ps_c[:, 0:N])
                    p2.append(p)
                for m in range(KC):
                    ps_q = po.tile([128, 512], f32, tag="po")
                    for kk in range(2):
                        mm_dr(
                            ps_q[:, 0:NL],
                            wtile(wpp8, m)[:, 2 * kk : 2 * kk + 2, :],
                            at8[:, 2 * kk : 2 * kk + 2, 0:NL],
                            start=(kk == 0), stop=(kk == 1),
                        )
                    pq = gpool.tile([128, 72], fp8, tag=f"pqb8_{m}")
                    nc.vector.tensor_scalar(
                        pq[:, 0:NL], ps_q[:, 0:NL], b1s[:, m : m + 1], None,
                        op0=Alu.add,
                    )
                    nc.vector.memset(pq[:, NL:72], 0.0)
                    pqb8.append(pq)

                def p2win(m, q0):
                    base = p2[m][:, 0:1]
                    return bass.AP(
                        tensor=base.tensor, offset=base.offset + q0,
                        ap=[list(base.ap[0]), [16, 2], [1, QR], [1, TW]],
                    )

                def pqbwin(m, q0):
                    base = pqb8[m][:, 0:1]
                    return bass.AP(
                        tensor=base.tensor, offset=base.offset + q0,
                        ap=[list(base.ap[0]), [4, 2], [1, QR], [0, TW]],
                    )

                # ---- main loop over row-quads ----
                psw2 = [None]
                for q in range(NQ):
                    i0 = QR * q
                    absq = work.tile([128, KC, QR * TW], fp8, tag="absq")
                    dsub = work.tile(
                        [128, max(POOL_CHUNKS, 1), QR * TW], bf16, tag="dsub"
                    )
                    for k in range(KC):
                        if k < POOL_CHUNKS:
                            base = at[:, k, 0:1]
                            j_ap = bass.AP(
                                tensor=base.tensor, offset=base.offset + i0,
                                ap=[list(base.ap[0]), [1, QR], [1, TW]],
                            )
                            i_ap = bass.AP(
                                tensor=base.tensor, offset=base.offset + i0,
                                ap=[list(base.ap[0]), [1, QR], [0, TW]],
                            )
                            nc.gpsimd.tensor_tensor(
                                dsub[:, k, :], j_ap, i_ap, op=Alu.subtract
                            )
                            nc.vector.tensor_scalar(
                                absq[:, k, :], dsub[:, k, :], 0.0, None,
                                op0=Alu.max,
                            )
                        else:
                            for r in range(QR):
                                i = i0 + r
                                nc.vector.tensor_scalar(
                                    absq[:, k, TW * r : TW * (r + 1)],
                                    at[:, k, i : i + TW],
                                    ai32[:, k, i : i + 1],
                                    0.0, op0=Alu.subtract, op1=Alu.max,
                                )

                    hh = work.tile([128, KC * 512], bf16, tag="hh")
                    for mm in range(2):
                        ps_h = ph.tile([128, 1024], f32, tag="ph")
                        for mi, m in enumerate((2 * mm, 2 * mm + 1)):
                            bank = ps_h[:, 512 * mi : 512 * (mi + 1)]
                            mm_dr(bank, id8_3d, p2win(m, i0),
                                  start=True, stop=False)
                            mm_dr(bank, id8_3d, pqbwin(m, i0),
                                  start=False, stop=False)
                            for kk in range(2):
                                mm_dr(
                                    bank,
                                    wtile(w1b8, m)[:, 2 * kk : 2 * kk + 2, :],
                                    absq[:, 2 * kk : 2 * kk + 2, :],
                                    start=False, stop=(kk == 1),
                                )
                        nc.scalar.activation(
                            hh[:, 1024 * mm : 1024 * (mm + 1)], ps_h[:],
                            ACTF, scale=1.0 / WS,
                        )

                    g_, s = q // 3, q % 3
                    if s == 0:
                        psw2[0] = po.tile([103, 512], f32, tag="po", name="psw2")
                        if sim_compat:
                            nc.vector.memset(psw2[0][:], 0.0)
                    for k in range(KC):
                        nc.tensor.matmul(
                            psw2[0][32 * s : 32 * s + C, :],
                            w2sb[:, C * k : C * (k + 1)],
                            hh[:, 512 * k : 512 * (k + 1)],
                            start=(k == 0), stop=(k == KC - 1),
                        )
                    if s == 2 or q == NQ - 1:
                        nparts = 32 * s + C
                        tmp = opool.tile([103, 512], f32, tag="tmp")
                        nc.vector.tensor_scalar(
                            tmp[0:nparts, :], psw2[0][0:nparts, :],
                            b2r[0:nparts, :], None, op0=Alu.add,
                        )
                        for s_ in range(s + 1):
                            nc.sync.dma_start(
                                out1_d.ap()[3 * g_ + s_, :, :],
                                tmp[32 * s_ : 32 * s_ + C, :],
                            )

                # ---- antipodal pass: pairs (i, i+128), i in 0..63 ----
                absA = work.tile([128, KC, NL], fp8, tag="absA")
                dA = work.tile([128, KC, NL], bf16, tag="dA")
                for k in range(KC):
                    nc.vector.tensor_tensor(
                        dA[:, k, :], at[:, k, TW : TW + NL], at[:, k, 0:NL],
                        op=Alu.subtract,
                    )
                    nc.vector.tensor_scalar(
                        absA[:, k, :], dA[:, k, :], 0.0, None, op0=Alu.max
                    )
                hhA = work.tile([128, KC * NL], bf16, tag="hhA")
                for m in range(KC):
                    ps_a = po.tile([128, 512], f32, tag="po")
                    bank = ps_a[:, 0:NL]
                    jsrc = p2[m][:, 0:1]
                    j_ap = bass.AP(
                        tensor=jsrc.tensor, offset=jsrc.offset + TW,
                        ap=[list(jsrc.ap[0]), [64, 2], [1, NL]],
                    )
                    mm_dr(bank, id8_3d, j_ap, start=True, stop=False)
                    isrc = pqb8[m][:, 0:1]
                    i_ap = bass.AP(
                        tensor=isrc.tensor, offset=isrc.offset,
                        ap=[list(isrc.ap[0]), [4, 2], [1, NL]],
                    )
                    mm_dr(bank, id8_3d, i_ap, start=False, stop=False)
                    for kk in range(2):
                        mm_dr(
                            bank,
                            wtile(w1b8, m)[:, 2 * kk : 2 * kk + 2, :],
                            absA[:, 2 * kk : 2 * kk + 2, :],
                            start=False, stop=(kk == 1),
                        )
                    nc.scalar.activation(
                        hhA[:, NL * m : NL * (m + 1)], bank,
                        ACTF, scale=1.0 / WS,
                    )
                ps_o = po.tile([103, 512], f32, tag="po")
                for k in range(KC):
                    nc.tensor.matmul(
                        ps_o[0:C, 0:NL],
                        w2sb[:, C * k : C * (k + 1)],
                        hhA[:, NL * k : NL * (k + 1)],
                        start=(k == 0), stop=(k == KC - 1),
                    )
                tmpA = opool.tile([C, NL], f32, tag="tmpA")
                nc.vector.tensor_scalar(
                    tmpA[:], ps_o[0:C, 0:NL], b2r[0:C, :], None, op0=Alu.add
                )
                nc.sync.dma_start(out2_d.ap(), tmpA[:])

            for _ in range(reps):
                body()

    nc.compile()
    return nc


def _get(reps=1, sim_compat=False):
    key = (reps, sim_compat)
    if key not in _CACHE:
        _CACHE[key] = _build(reps, sim_compat)
    return _CACHE[key]


def _prep_weights(W1, b1, W2, b2):
    """Host-side weight packing. Device tile layout per m-block (rows
    128m..128m+127 of the DRAM tensor): tile[p, 128k+f] = w[128k+p, 128m+f],
    i.e. contraction chunk k as weight slot k, output feature f."""
    W1 = np.asarray(W1, np.float32)
    W1a, W1b = W1[0:D], W1[D : 2 * D]

    def pack(w):
        out = np.empty((D, D), np.float32)
        for m in range(KC):
            for k in range(KC):
                out[128 * m : 128 * (m + 1), 128 * k : 128 * (k + 1)] = w[
                    128 * k : 128 * (k + 1), 128 * m : 128 * (m + 1)
                ]
        return out

    clip8 = lambda x: np.clip(x, -240.0, 240.0).astype(F8)
    w1b8 = clip8(pack(2 * WS * W1b))
    wpm8 = clip8(pack(WS * (W1a - W1b)))
    wpp8 = clip8(pack(WS * (W1a + W1b)))
    W2f = np.asarray(W2, np.float32)
    w2b = np.zeros((128, KC * C), np.float32)
    for k in range(KC):
        w2b[:, C * k : C * (k + 1)] = W2f[128 * k : 128 * (k + 1), :]
    w2b = w2b.astype(BF)
    b1f = np.asarray(b1, np.float32)
    b1s = np.zeros((128, KC), np.float32)
    for m in range(KC):
        b1s[:, m] = WS * b1f[128 * m : 128 * (m + 1)]
    b2f = np.asarray(b2, np.float32)
    b2r = np.zeros((128, 1), np.float32)
    for s in range(3):
        b2r[32 * s : 32 * s + C, 0] = b2f
    id8 = np.zeros((128, 256), np.float32)
    id8[:, 0:128] = np.eye(128)
    id8 = id8.astype(F8)
    return w1b8, wpm8, wpp8, w2b, b1s, b2r, id8


def _shard_inputs(hidden_states, W1, b1, W2, b2, atom_indices):
    hs = np.asarray(hidden_states, np.float32)
    idx = np.clip(np.asarray(atom_indices).astype(np.int64), 0, T - 1)
    w1b8, wpm8, wpp8, w2b, b1s, b2r, id8 = _prep_weights(W1, b1, W2, b2)
    in_maps = []
    for c in range(NCORES):
        b = c // RB
        r0 = NL * (c % RB)
        idx_roll = np.roll(idx[b], -r0).astype(np.int32).reshape(2, 128).T
        in_maps.append(
            {
                "h": hs[b].astype(BF),
                "idx": np.ascontiguousarray(idx_roll),
                "w1b8": w1b8, "wpm8": wpm8, "wpp8": wpp8,
                "w2b": w2b, "b1s": b1s, "b2r": b2r, "id8": id8,
            }
        )
    return in_maps


def _unshard(results, atom_mask):
    full = np.empty((B, C, N, N), np.float32)
    for c in range(NCORES):
        b = c // RB
        r0 = NL * (c % RB)
        o1 = results[c]["out1"]  # [16, 7, 512]
        o2 = results[c]["out2"]  # [7, 64]
        blk = np.empty((C, NL, TW + 1), np.float32)
        blk[:, :, 0:TW] = (
            o1.reshape(NQ, C, QR, TW).transpose(1, 0, 2, 3).reshape(C, NL, TW)
        )
        blk[:, :, TW] = o2
        rows = r0 + np.arange(NL)
        idx_j = (rows[:, None] + np.arange(TW + 1)[None, :]) % N
        np.put_along_axis(
            full[b, :, r0 : r0 + NL, :],
            np.broadcast_to(idx_j[None], (C, NL, TW + 1)),
            blk,
            axis=2,
        )
    offs = (np.arange(N)[None, :] - np.arange(N)[:, None]) % N
    low = offs > TW
    fullT = np.transpose(full, (0, 1, 3, 2))
    full = np.where(low[None, None], fullT, full)
    di = np.arange(N)
    full[:, :, di, di] = MASK_FILL
    mask = np.asarray(atom_mask).astype(bool)
    if not mask.all():
        valid = mask[:, :, None] & mask[:, None, :]
        valid &= ~np.eye(N, dtype=bool)[None]
        full = np.where(valid[:, None, :, :], full, np.float32(MASK_FILL))
    return full


def kernel(hidden_states, W1, b1, W2, b2, atom_indices, atom_mask):
    from concourse.bass_utils import run_bass_kernel_spmd

    nc = _get(1)
    in_maps = _shard_inputs(hidden_states, W1, b1, W2, b2, atom_indices)
    res = run_bass_kernel_spmd(nc, in_maps, list(range(NCORES)))
    return _unshard(res.results, atom_mask)


# revision 4
# speedup vs baseline: 2.3751x; 1.1451x over previous
"""Trainium2 Bass kernel for nn_BondPredictor (gnn_message_passing) — v3.

Computes, for each batch b:
    A      = hidden_states[b][clip(atom_indices[b])]          # [256, 512] gather
    pair   = concat(A[i]+A[j], |A[i]-A[j]|)                   # [256,256,1024]
    h      = gelu(pair @ W1 + b1)                             # [256,256,512]
    logits = h @ W2 + b2  -> [7, 256, 256], diagonal = -10000

Sharding: 8 cores = 2 batches x 4 row-blocks of 64 rows; atom axis rolled by
-64*(c%4) per core (pure SPMD); host un-rolls on unshard. Pair symmetry:
each row computes cyclic offsets (j-i) mod 256 in [0,128]; the host mirrors
offsets 129..255 from the transpose.

v3 engine plan (vs the bf16 v2 baseline at ~130us):
- Gather via indirect DMA + DMA transposes (no one-hot matmuls).
- |d| = 2 relu(d) - d; relu features quantized to fp8e4 by DVE dual-op
  tensor_scalar (sub,max) at the 2x SBUF rate; the per-pair contraction runs
  as fp8 DoubleRow matmuls (2x PE), first-layer weights prescaled x128 on
  the host (x64 fp8-range scale, x2 from the relu identity); the gelu
  activation applies scale=1/64 to descale.
- Rank-one injections ws*(P-Q)[j] and ws*(P+Q+b1)[i] enter PSUM through
  identity-weight DoubleRow matmuls whose moving operand is a 4-D strided
  AP (overlapping column windows for the j-term; inner stride-0 broadcast
  for the i-term). Zero per-quad DVE cost for the rank-one terms.
- P/Q phase also fp8 DoubleRow with host-prepped ws*(W1a-+W1b) weights.
- Second layer bf16; outputs of 3 quads packed into one PSUM bank at
  partition offsets 0/32/64 so the b2-add drain is one DVE op per 3 quads.
- Diagonal fill and the symmetry mirror happen on the host.
"""

import sys

sys.path.insert(0, "/opt/trn_rl_repo")

import numpy as np
import ml_dtypes

F8 = ml_dtypes.float8_e4m3
BF = ml_dtypes.bfloat16

B, T, D, N, C = 2, 1024, 512, 256, 7
NCORES = 8
RB = 4                # row-blocks per batch
NL = N // RB          # 64 rows per core
QR = 4                # rows per quad
NQ = NL // QR         # 16 quads
KC = D // 128         # 4 chunks of the 512-dim contraction
TW = 128              # cols per row: cyclic offsets (j-i) in [0,127]
MASK_FILL = -10000.0
WS = 64.0             # fp8 weight prescale

POOL_CHUNKS = 1       # absq feature-chunks whose subtract runs on GpSimd

_CACHE = {}


def _build(reps=1, sim_compat=False):
    import concourse.bass as bass
    import concourse.bacc as bacc
    import concourse.tile as tile
    from concourse import mybir

    f32 = mybir.dt.float32
    bf16 = mybir.dt.bfloat16
    fp8 = mybir.dt.float8e4
    i32 = mybir.dt.int32
    Alu = mybir.AluOpType
    Act = mybir.ActivationFunctionType
    DR = mybir.MatmulPerfMode.DoubleRow
    ACTF = Act.Relu if sim_compat else Act.Gelu

    nc = bacc.Bacc("TRN2", target_bir_lowering=False, debug=False)

    h_d = nc.dram_tensor("h", [T, D], bf16, kind="ExternalInput")
    idx_d = nc.dram_tensor("idx", [128, 2], i32, kind="ExternalInput")
    w1b8_d = nc.dram_tensor("w1b8", [D, D], fp8, kind="ExternalInput")
    wpm8_d = nc.dram_tensor("wpm8", [D, D], fp8, kind="ExternalInput")
    wpp8_d = nc.dram_tensor("wpp8", [D, D], fp8, kind="ExternalInput")
    w2b_d = nc.dram_tensor("w2b", [128, KC * C], bf16, kind="ExternalInput")
    b1s_d = nc.dram_tensor("b1s", [128, KC], f32, kind="ExternalInput")
    b2r_d = nc.dram_tensor("b2r", [128, 1], f32, kind="ExternalInput")
    id8_d = nc.dram_tensor("id8", [128, 256], fp8, kind="ExternalInput")
    out1_d = nc.dram_tensor("out1", [NQ, C, QR * TW], f32, kind="ExternalOutput")
    out2_d = nc.dram_tensor("out2", [C, NL], f32, kind="ExternalOutput")

    with tile.TileContext(nc) as tc:
        from contextlib import ExitStack

        with ExitStack() as ctx:
            const = ctx.enter_context(tc.tile_pool(name="const", bufs=1))
            gpool = ctx.enter_context(tc.tile_pool(name="g", bufs=2))
            work = ctx.enter_context(tc.tile_pool(name="work", bufs=3))
            opool = ctx.enter_context(tc.tile_pool(name="o", bufs=2))
            # PSUM: ph 2 bufs x 2 banks + po_c 2 x 1 + po_w 2 x 1 = 8 banks
            ph = ctx.enter_context(
                tc.tile_pool(name="ph", bufs=2, space=bass.MemorySpace.PSUM)
            )
            po_c = ctx.enter_context(
                tc.tile_pool(name="po_c", bufs=2, space=bass.MemorySpace.PSUM)
            )
            po_w = ctx.enter_context(
                tc.tile_pool(name="po_w", bufs=2, space=bass.MemorySpace.PSUM)
            )

            # ---- one-time constants ----
            id8 = const.tile([128, 256], fp8, tag="id8")
            nc.sync.dma_start(id8[:], id8_d.ap())
            id8_3d = id8[:].rearrange("p (s m) -> p s m", s=2)
            idb = const.tile([128, 128], bf16, tag="idb")
            nc.vector.tensor_copy(idb[:], id8[:, 0:128])

            def slot(ap_, sl):
                dims = [list(d) for d in ap_.ap]
                return bass.AP(
                    tensor=ap_.tensor,
                    offset=ap_.offset + sl * dims[1][0],
                    ap=[dims[0]] + dims[2:],
                )

            def mm_dr(out, lhsT, rhs, start, stop):
                """DoubleRow matmul; in sim_compat, lower to interp-friendly
                non-DR matmuls (slot loop) with identical operands/deps."""
                if not sim_compat:
                    nc.tensor.matmul(out, lhsT, rhs, start=start, stop=stop,
                                     perf_mode=DR)
                    return
                for sl in range(2):
                    nc.tensor.matmul(out, slot(lhsT, sl), slot(rhs, sl),
                                     start=(start and sl == 0),
                                     stop=(stop and sl == 1))
            w1b8 = const.tile([128, KC * 512], fp8, tag="w1b8")
            wpm8 = const.tile([128, KC * 512], fp8, tag="wpm8")
            wpp8 = const.tile([128, KC * 512], fp8, tag="wpp8")
            for m in range(KC):
                sl = slice(512 * m, 512 * (m + 1))
                rows = slice(128 * m, 128 * (m + 1))
                nc.sync.dma_start(w1b8[:, sl], w1b8_d.ap()[rows, :])
                nc.sync.dma_start(wpm8[:, sl], wpm8_d.ap()[rows, :])
                nc.sync.dma_start(wpp8[:, sl], wpp8_d.ap()[rows, :])
            w2sb = const.tile([128, KC * C], bf16, tag="w2sb")
            nc.sync.dma_start(w2sb[:], w2b_d.ap())
            b1s = const.tile([128, KC], f32, tag="b1s")
            nc.sync.dma_start(b1s[:], b1s_d.ap())
            b2r = const.tile([128, 1], f32, tag="b2r")
            nc.sync.dma_start(b2r[:], b2r_d.ap())

            def wtile(t, m):
                return t[:, 512 * m : 512 * (m + 1)].rearrange(
                    "p (k f) -> p k f", k=KC
                )

            def prep():
                # ---- gather A = h[idx] (atom-major), transpose to f-major ----
                idx_sb = gpool.tile([128, 2], i32, tag="idx_sb")
                nc.sync.dma_start(idx_sb[:], idx_d.ap())
                ga = []
                for t_ in range(2):
                    g = gpool.tile([128, D], bf16, tag=f"ga{t_}")
                    nc.gpsimd.indirect_dma_start(
                        out=g[:], out_offset=None, in_=h_d.ap(),
                        in_offset=bass.IndirectOffsetOnAxis(
                            ap=idx_sb[:, t_ : t_ + 1], axis=0
                        ),
                    )
                    ga.append(g)
                at = gpool.tile([128, KC, N], bf16, tag="at")
                for t_ in range(2):
                    for k in range(KC):
                        eng = nc.sync if (k % 2 == 0) else nc.scalar
                        eng.dma_start_transpose(
                            at[:, k, 128 * t_ : 128 * (t_ + 1)],
                            ga[t_][:, 128 * k : 128 * (k + 1)],
                        )
                at8 = gpool.tile([128, KC, N], fp8, tag="at8")
                nc.vector.tensor_copy(
                    at8[:].rearrange("p k a -> p (k a)"),
                    at[:].rearrange("p k a -> p (k a)"),
                )
                ai32 = gpool.tile([128, KC, NL], f32, tag="ai32")
                nc.vector.tensor_copy(
                    ai32[:].rearrange("p k a -> p (k a)"), at[:, :, 0:NL]
                )

                # ---- P/Q phase: p2 = ws(P-Q) all atoms, pqb8 = ws(P+Q+b1)[0:64]
                p2, pqb8 = [], []
                for m in range(KC):
                    ps_c = po_c.tile([128, 512], f32, tag="po_c")
                    for kk in range(2):
                        mm_dr(
                            ps_c[:, 0:N],
                            wtile(wpm8, m)[:, 2 * kk : 2 * kk + 2, :],
                            at8[:, 2 * kk : 2 * kk + 2, :],
                            start=(kk == 0), stop=(kk == 1),
                        )
                    p = gpool.tile([128, N], fp8, tag=f"p2_{m}")
                    nc.vector.tensor_copy(p[:], ps_c[:, 0:N])
                    p2.append(p)
                for m in range(KC):
                    ps_q = po_c.tile([128, 512], f32, tag="po_c")
                    for kk in range(2):
                        mm_dr(
                            ps_q[:, 0:NL],
                            wtile(wpp8, m)[:, 2 * kk : 2 * kk + 2, :],
                            at8[:, 2 * kk : 2 * kk + 2, 0:NL],
                            start=(kk == 0), stop=(kk == 1),
                        )
                    pq = gpool.tile([128, 72], fp8, tag=f"pqb8_{m}")
                    nc.vector.tensor_scalar(
                        pq[:, 0:NL], ps_q[:, 0:NL], b1s[:, m : m + 1], None,
                        op0=Alu.add,
                    )
                    nc.vector.memset(pq[:, NL:72], 0.0)
                    pqb8.append(pq)
                return at, ai32, p2, pqb8

            def main(st):
                at, ai32, p2, pqb8 = st

                def p2win(m, q0):
                    base = p2[m][:, 0:1]
                    return bass.AP(
                        tensor=base.tensor, offset=base.offset + q0,
                        ap=[list(base.ap[0]), [16, 2], [1, QR], [1, TW]],
                    )

                def pqbwin(m, q0):
                    base = pqb8[m][:, 0:1]
                    return bass.AP(
                        tensor=base.tensor, offset=base.offset + q0,
                        ap=[list(base.ap[0]), [4, 2], [1, QR], [0, TW]],
                    )

                # ---- antipodal pass: pairs (i, i+128), i in 0..63 ----
                absA = work.tile([128, KC, NL], fp8, tag="absA")
                dA = work.tile([128, KC, NL], bf16, tag="dA")
                for k in range(KC):
                    nc.vector.tensor_tensor(
                        dA[:, k, :], at[:, k, TW : TW + NL], at[:, k, 0:NL],
                        op=Alu.subtract,
                    )
                    nc.vector.tensor_scalar(
                        absA[:, k, :], dA[:, k, :], 0.0, None, op0=Alu.max
                    )
                hhA = work.tile([128, KC * NL], bf16, tag="hhA")
                for m in range(KC):
                    ps_a = po_w.tile([128, 512], f32, tag="po_w")
                    bank = ps_a[:, 0:NL]
                    jsrc = p2[m][:, 0:1]
                    j_ap = bass.AP(
                        tensor=jsrc.tensor, offset=jsrc.offset + TW,
                        ap=[list(jsrc.ap[0]), [64, 2], [1, NL]],
                    )
                    mm_dr(bank, id8_3d, j_ap, start=True, stop=False)
                    isrc = pqb8[m][:, 0:1]
                    i_ap = bass.AP(
                        tensor=isrc.tensor, offset=isrc.offset,
                        ap=[list(isrc.ap[0]), [4, 2], [1, NL]],
                    )
                    mm_dr(bank, id8_3d, i_ap, start=False, stop=False)
                    for kk in range(2):
                        mm_dr(
                            bank,
                            wtile(w1b8, m)[:, 2 * kk : 2 * kk + 2, :],
                            absA[:, 2 * kk : 2 * kk + 2, :],
                            start=False, stop=(kk == 1),
                        )
                    nc.scalar.activation(
                        hhA[:, NL * m : NL * (m + 1)], bank,
                        ACTF, scale=1.0 / WS,
                    )
                ps_o = po_w.tile([128, 512], f32, tag="po_w")
                for k in range(KC):
                    nc.tensor.matmul(
                        ps_o[0:C, 0:NL],
                        w2sb[:, C * k : C * (k + 1)],
                        hhA[:, NL * k : NL * (k + 1)],
                        start=(k == 0), stop=(k == KC - 1),
                    )
                tmpA = opool.tile([C, NL], f32, tag="tmpA")
                nc.vector.tensor_scalar(
                    tmpA[:], ps_o[0:C, 0:NL], b2r[0:C, :], None, op0=Alu.add
                )
                nc.sync.dma_start(out2_d.ap(), tmpA[:])

                # ---- main loop over row-quads ----
                psw2 = [None]
                for q in range(NQ):
                    i0 = QR * q
                    absq = work.tile([128, KC, QR * TW], fp8, tag="absq")
                    dsub = work.tile(
                        [128, max(POOL_CHUNKS, 1), QR * TW], bf16, tag="dsub"
                    )
                    for k in range(KC):
                        if k < POOL_CHUNKS:
                            base = at[:, k, 0:1]
                            j_ap = bass.AP(
                                tensor=base.tensor, offset=base.offset + i0,
                                ap=[list(base.ap[0]), [1, QR], [1, TW]],
                            )
                            i_ap = bass.AP(
                                tensor=base.tensor, offset=base.offset + i0,
                                ap=[list(base.ap[0]), [1, QR], [0, TW]],
                            )
                            nc.gpsimd.tensor_tensor(
                                dsub[:, k, :], j_ap, i_ap, op=Alu.subtract
                            )
                            nc.vector.tensor_scalar(
                                absq[:, k, :], dsub[:, k, :], 0.0, None,
                                op0=Alu.max,
                            )
                        else:
                            for r in range(QR):
                                i = i0 + r
                                nc.vector.tensor_scalar(
                                    absq[:, k, TW * r : TW * (r + 1)],
                                    at[:, k, i : i + TW],
                                    ai32[:, k, i : i + 1],
                                    0.0, op0=Alu.subtract, op1=Alu.max,
                                )

                    hh = work.tile([128, KC * 512], bf16, tag="hh")
                    for mm in range(2):
                        ps_h = ph.tile([128, 1024], f32, tag="ph")
                        for mi, m in enumerate((2 * mm, 2 * mm + 1)):
                            bank = ps_h[:, 512 * mi : 512 * (mi + 1)]
                            mm_dr(bank, id8_3d, p2win(m, i0),
                                  start=True, stop=False)
                            mm_dr(bank, id8_3d, pqbwin(m, i0),
                                  start=False, stop=False)
                            for kk in range(2):
                                mm_dr(
                                    bank,
                                    wtile(w1b8, m)[:, 2 * kk : 2 * kk + 2, :],
                                    absq[:, 2 * kk : 2 * kk + 2, :],
                                    start=False, stop=(kk == 1),
                                )
                        nc.scalar.activation(
                            hh[:, 1024 * mm : 1024 * (mm + 1)], ps_h[:],
                            ACTF, scale=1.0 / WS,
                        )

                    g_, s = q // 3, q % 3
                    if s == 0:
                        psw2[0] = po_w.tile([128, 512], f32, tag="po_w", name="psw2")
                        if sim_compat:
                            nc.vector.memset(psw2[0][:], 0.0)
                    for k in range(KC):
                        nc.tensor.matmul(
                            psw2[0][32 * s : 32 * s + C, :],
                            w2sb[:, C * k : C * (k + 1)],
                            hh[:, 512 * k : 512 * (k + 1)],
                            start=(k == 0), stop=(k == KC - 1),
                        )
                    if s == 2 or q == NQ - 1:
                        nparts = 32 * s + C
                        tmp = opool.tile([103, 512], f32, tag="tmp")
                        nc.vector.tensor_scalar(
                            tmp[0:nparts, :], psw2[0][0:nparts, :],
                            b2r[0:nparts, :], None, op0=Alu.add,
                        )
                        for s_ in range(s + 1):
                            nc.sync.dma_start(
                                out1_d.ap()[3 * g_ + s_, :, :],
                                tmp[32 * s_ : 32 * s_ + C, :],
                            )


            st = prep()
            for r_ in range(reps):
                nxt = prep() if r_ + 1 < reps else None
                main(st)
                st = nxt

    nc.compile()
    return nc


def _get(reps=1, sim_compat=False):
    key = (reps, sim_compat)
    if key not in _CACHE:
        _CACHE[key] = _build(reps, sim_compat)
    return _CACHE[key]


def _prep_weights(W1, b1, W2, b2):
    """Host-side weight packing. Device tile layout per m-block (rows
    128m..128m+127 of the DRAM tensor): tile[p, 128k+f] = w[128k+p, 128m+f],
    i.e. contraction chunk k as weight slot k, output feature f."""
    W1 = np.asarray(W1, np.float32)
    W1a, W1b = W1[0:D], W1[D : 2 * D]

    def pack(w):
        out = np.empty((D, D), np.float32)
        for m in range(KC):
            for k in range(KC):
                out[128 * m : 128 * (m + 1), 128 * k : 128 * (k + 1)] = w[
                    128 * k : 128 * (k + 1), 128 * m : 128 * (m + 1)
                ]
        return out

    clip8 = lambda x: np.clip(x, -240.0, 240.0).astype(F8)
    w1b8 = clip8(pack(2 * WS * W1b))
    wpm8 = clip8(pack(WS * (W1a - W1b)))
    wpp8 = clip8(pack(WS * (W1a + W1b)))
    W2f = np.asarray(W2, np.float32)
    w2b = np.zeros((128, KC * C), np.float32)
    for k in range(KC):
        w2b[:, C * k : C * (k + 1)] = W2f[128 * k : 128 * (k + 1), :]
    w2b = w2b.astype(BF)
    b1f = np.asarray(b1, np.float32)
    b1s = np.zeros((128, KC), np.float32)
    for m in range(KC):
        b1s[:, m] = WS * b1f[128 * m : 128 * (m + 1)]
    b2f = np.asarray(b2, np.float32)
    b2r = np.zeros((128, 1), np.float32)
    for s in range(3):
        b2r[32 * s : 32 * s + C, 0] = b2f
    id8 = np.zeros((128, 256), np.float32)
    id8[:, 0:128] = np.eye(128)
    id8 = id8.astype(F8)
    return w1b8, wpm8, wpp8, w2b, b1s, b2r, id8


def _shard_inputs(hidden_states, W1, b1, W2, b2, atom_indices):
    hs = np.asarray(hidden_states, np.float32)
    idx = np.clip(np.asarray(atom_indices).astype(np.int64), 0, T - 1)
    w1b8, wpm8, wpp8, w2b, b1s, b2r, id8 = _prep_weights(W1, b1, W2, b2)
    in_maps = []
    for c in range(NCORES):
        b = c // RB
        r0 = NL * (c % RB)
        idx_roll = np.roll(idx[b], -r0).astype(np.int32).reshape(2, 128).T
        in_maps.append(
            {
                "h": hs[b].astype(BF),
                "idx": np.ascontiguousarray(idx_roll),
                "w1b8": w1b8, "wpm8": wpm8, "wpp8": wpp8,
                "w2b": w2b, "b1s": b1s, "b2r": b2r, "id8": id8,
            }
        )
    return in_maps


def _unshard(results, atom_mask):
    full = np.empty((B, C, N, N), np.float32)
    for c in range(NCORES):
        b = c // RB
        r0 = NL * (c % RB)
        o1 = results[c]["out1"]  # [16, 7, 512]
        o2 = results[c]["out2"]  # [7, 64]
        blk = np.empty((C, NL, TW + 1), np.float32)
        blk[:, :, 0:TW] = (
            o1.reshape(NQ, C, QR, TW).transpose(1, 0, 2, 3).reshape(C, NL, TW)
        )
        blk[:, :, TW] = o2
        rows = r0 + np.arange(NL)
        idx_j = (rows[:, None] + np.arange(TW + 1)[None, :]) % N
        np.put_along_axis(
            full[b, :, r0 : r0 + NL, :],
            np.broadcast_to(idx_j[None], (C, NL, TW + 1)),
            blk,
            axis=2,
        )
    offs = (np.arange(N)[None, :] - np.arange(N)[:, None]) % N
    low = offs > TW
    fullT = np.transpose(full, (0, 1, 3, 2))
    full = np.where(low[None, None], fullT, full)
    di = np.arange(N)
    full[:, :, di, di] = MASK_FILL
    mask = np.asarray(atom_mask).astype(bool)
    if not mask.all():
        valid = mask[:, :, None] & mask[:, None, :]
        valid &= ~np.eye(N, dtype=bool)[None]
        full = np.where(valid[:, None, :, :], full, np.float32(MASK_FILL))
    return full


def kernel(hidden_states, W1, b1, W2, b2, atom_indices, atom_mask):
    from concourse.bass_utils import run_bass_kernel_spmd

    nc = _get(1)
    in_maps = _shard_inputs(hidden_states, W1, b1, W2, b2, atom_indices)
    res = run_bass_kernel_spmd(nc, in_maps, list(range(NCORES)))
    return _unshard(res.results, atom_mask)
